# revision 29
# baseline (speedup 1.0000x reference)
"""Trainium2 Bass kernel for nn_CLModel_6863357739229 (vq_codebook).

Model (per batch row x of dim D=1024):
  for s in 0..3:  feats_s = adapter_s(x) = (relu(LN(x @ W1[s] + b1)) @ W2[s] + b2)   [L=256]
  cos[s,c]   = <normed(feats_s), normed(anc[s,c])>          (anchors precomputed on host)
  probs_s    = softmax_c(5 * cos[s,:])                      (temp 0.1, affine shift dropped)
  w          = softmax_s(relu(LN(x @ Wg1 + bg1)) @ Wg2 + bg2)
  out        = log(sum_s w_s * probs_s + 1e-8)              [C=7]

Strategy: data-parallel over batch across 8 NeuronCores (4096 rows/core).
Weights resident in SBUF, matmuls bf16 (fp32 PSUM), tails fp32.

Fast path exploits two identities (valid because b1=0, g1=1, be1=0, b2=0
for this model instance):
  1) LN's mean subtraction is linear in x, so it folds into the weights:
     W1' = W1 - rowwise-mean -> x @ W1' = h - mu directly.
  2) LN's 1/std scale is a positive per-row scalar that commutes through
     relu and the (linear) second layer, and the downstream cosine /
     norm ratio is scale-invariant -> the variance/rstd is never needed.
  So adapter(x) reduces to relu(x @ W1') @ W2 up to per-row scale.

Layer 1 is computed TRANSPOSED (W1' chunks stationary, X^T moving, four
128-row tiles batched to N=512 streams): H'^T comes out of PSUM directly
in the [feature, batch] orientation that layer 2 needs for its stationary
operand, eliminating all 32 per-tile PE transposes + their PSUM->SBUF
evictions; relu is applied during the single PSUM->SBUF eviction pass.
The gate branch (softmax over domains is NOT scale-invariant) keeps the
full LayerNorm. rsqrt is computed as exp(-0.5*ln(x)) to stay inside the
single ACT table set {exp, ln, relu, square, copy} (no table switches).
"""

import numpy as np
import ml_dtypes

# ---- model/shard dims (hardcoded; harness contract) ----
B, D, S, C, L = 32768, 1024, 4, 7, 256
P = 128                  # SBUF partitions
NCORES = 8
BPC = B // NCORES        # rows per core = 4096
NT = BPC // P            # batch tiles per core = 32
GT = 4                   # batch tiles per L1 group (N=512 streams)
NG = NT // GT            # groups per core = 8
KD = D // P              # contraction chunks over D = 8
DH = D // 2              # gate hidden = 512
NF = L                   # 256 F columns
NCAT = NF + C + 1        # F + G + pad = 264
EPS = 1e-8
LN_EPS = 1e-5
TEMP = 0.1

BF16 = ml_dtypes.bfloat16
F8E4 = ml_dtypes.float8_e4m3

# Layer-1 adapter matmuls in fp8 DoubleRow (2 MACs/cell/cycle, half the
# matmul count). Per-element quantization error ~2% washes out to ~1e-3
# at the output through the scale-invariant cosine path; the gate branch
# (whose errors hit the output directly) stays bf16.
FP8_L1 = True
FP8_L2 = True
XSC = 16.0     # fp8 scale for X (values ~N(0,1) -> +-80, e4m3 max 240)
WSC = 64.0     # fp8 scale for W1' (values ~N(0,1/1024) -> +-10)
W2SC = 64.0    # fp8 scale for W2c
HSC = 1.0 / 32.0   # eviction scale for Hn (|h''| ~ 1024*|h-mu| -> +-128)
NCATP = 272    # fp8-L2 padded width (DoubleRow rhs Ko step must be %16 bytes)

_CACHE = {}


# --------------------------------------------------------------------------
# host-side preprocessing
# --------------------------------------------------------------------------

def _host_prepare(inputs):
    X = np.ascontiguousarray(np.asarray(inputs["mask_outputs"], dtype=np.float32))
    assert X.shape == (B, D), X.shape
    f64 = lambda k: np.asarray(inputs[k], dtype=np.float64)
    anchor = f64("emo_anchor")          # [C, S, D]
    W1, b1 = f64("W1"), f64("b1")       # [S,D,D], [S,D]
    g1, be1 = f64("g1"), f64("be1")     # [S,D]
    W2, b2 = f64("W2"), f64("b2")       # [S,D,L], [S,L]
    Wg1, bg1 = f64("Wg1"), f64("bg1")   # [D,DH], [DH]
    gg, bgb = f64("gg"), f64("bgb")     # [DH]
    Wg2, bg2 = f64("Wg2"), f64("bg2")   # [DH,S], [S]

    # anchors through the adapters, normalized — tiny, fp64 on host
    An = np.empty((S, C, L))
    for s in range(S):
        h = anchor[:, s, :] @ W1[s] + b1[s]
        mu = h.mean(-1, keepdims=True)
        v = np.square(h - mu).mean(-1, keepdims=True)
        h = np.maximum((h - mu) / np.sqrt(v + LN_EPS) * g1[s] + be1[s], 0.0)
        f = h @ W2[s] + b2[s]
        n = np.sqrt(np.square(f).sum(-1, keepdims=True))
        An[s] = f / np.maximum(n, EPS)

    # fold anchors into layer-2 weights: [W2 | W2@An^T | 0]
    W2c = np.zeros((S, D, NCAT), np.float64)
    gb = np.zeros((S, NCAT), np.float64)
    for s in range(S):
        W2c[s, :, :NF] = W2[s]
        W2c[s, :, NF:NF + C] = W2[s] @ An[s].T
        gb[s, :NF] = b2[s]
        gb[s, NF:NF + C] = b2[s] @ An[s].T

    flags = dict(
        b1=bool(np.any(b1 != 0.0)),
        aff1=bool(np.any(g1 != 1.0) or np.any(be1 != 0.0)),
        b2=bool(np.any(b2 != 0.0)),
        bg1=bool(np.any(bg1 != 0.0)),
        affg=bool(np.any(gg != 1.0) or np.any(bgb != 0.0)),
        bg2=bool(np.any(bg2 != 0.0)),
    )
    flags["fastln"] = not (flags["b1"] or flags["aff1"] or flags["b2"])
    flags["fp8"] = bool(flags["fastln"] and FP8_L1)
    flags["fp8l2"] = bool(flags["fp8"] and FP8_L2)

    arrays = {}
    Xb = X.astype(BF16)
    if flags["fastln"]:
        # mean-centered layer-1 weights (rank-1 fold of LN's mean-sub)
        W1p = W1 - W1.mean(axis=2, keepdims=True)          # [S, D, D]
        if flags["fp8"]:
            # DoubleRow layouts: contraction row r = dd*256 + ko*128 + ki
            # lhsT [ki, dd, e_k, ko, e_p], rhs [ki, g, dd, ko, b]
            w1q = (W1p * WSC).astype(F8E4)                 # [S, D, D]
            w1l = np.ascontiguousarray(
                w1q.reshape(S, KD // 2, 2, P, KD, P)
                .transpose(0, 3, 1, 4, 2, 5))              # [S,P,4,KD,2,P]
            xq = (X * XSC).astype(F8E4)
            arrays["_xf8_percore"] = (
                xq.reshape(NCORES, NG, GT * P, KD // 2, 2, P)
                .transpose(0, 5, 1, 3, 4, 2))              # [NC,P,NG,4,2,512]
        else:
            # stationary chunks [d_p, dk, ek, e_p]
            w1l = np.ascontiguousarray(
                W1p.reshape(S, KD, P, KD, P).transpose(0, 2, 1, 3, 4)).astype(BF16)
        # X^T per core, grouped GT tiles wide: [P(d_p), NG, KD(d_k), GT*P]
        arrays["_xt_percore"] = (
            Xb.reshape(NCORES, NG, GT * P, KD, P).transpose(0, 4, 1, 3, 2))
    else:
        # original orientation: [P(d_p), k, e]
        w1l = np.ascontiguousarray(
            W1.reshape(S, KD, P, D).transpose(0, 2, 1, 3)).astype(BF16)
        arrays["_xt_percore"] = (
            Xb.reshape(NCORES, NT, P, KD, P).transpose(0, 4, 1, 3, 2))
    if flags["fp8l2"]:
        W2cp = np.zeros((S, D, NCATP), np.float64)
        W2cp[:, :, :NCAT] = W2c * W2SC
        w2l = np.ascontiguousarray(
            W2cp.astype(F8E4)
            .reshape(S, KD // 2, 2, P, NCATP)
            .transpose(0, 3, 1, 2, 4))                     # [S,P,4,2,NCATP]
    else:
        w2l = np.ascontiguousarray(
            W2c.reshape(S, KD, P, NCAT).transpose(0, 2, 1, 3)).astype(BF16)
    wg1l = np.ascontiguousarray(
        Wg1.reshape(KD, P, DH).transpose(1, 0, 2)).astype(BF16)
    wg2l = np.ascontiguousarray(Wg2.T).astype(BF16)  # [S, DH] columns

    arrays.update({"w1": w1l, "w2c": w2l, "wg1": wg1l, "wg2": wg2l})
    if flags["b1"]:
        arrays["b1v"] = np.ascontiguousarray(b1).astype(BF16)
    if flags["aff1"]:
        arrays["g1v"] = np.ascontiguousarray(g1).astype(BF16)
        arrays["be1v"] = np.ascontiguousarray(be1).astype(BF16)
    if flags["b2"]:
        arrays["b2v"] = np.ascontiguousarray(gb).astype(np.float32)  # [S, NCAT]
    if flags["bg1"]:
        arrays["bg1v"] = np.ascontiguousarray(bg1).astype(np.float32)
    if flags["affg"]:
        arrays["ggv"] = np.ascontiguousarray(gg).astype(np.float32)
        arrays["bgbv"] = np.ascontiguousarray(bgb).astype(np.float32)
    if flags["bg2"]:
        arrays["bg2v"] = np.ascontiguousarray(bg2).astype(np.float32)
    return X, arrays, flags


# --------------------------------------------------------------------------
# device program
# --------------------------------------------------------------------------

def _build(flags, nt=NT, repeat=1):
    import concourse.bass as bass
    from concourse import bacc, mybir
    from concourse.tile import TileContext
    from concourse.masks import make_identity

    f32 = mybir.dt.float32
    bf16 = mybir.dt.bfloat16
    AF = mybir.ActivationFunctionType
    OP = mybir.AluOpType
    AX = mybir.AxisListType

    fastln = flags.get("fastln", False)
    fp8 = flags.get("fp8", False)
    ng = nt // GT
    assert (not fastln) or nt % GT == 0

    # All our ACT functions (Ln, Exp, Relu, Square, Identity/Copy) live in the
    # single 'natural_log_exp_and_others' table set, but bacc's greedy
    # set-chooser would ping-pong between 'natural_log' and 'exp_and_others'
    # (one ~2.7us table load per switch, ~18/tile). Restrict the chooser's
    # view so only the combined set claims these functions; set ids (dict
    # order) are preserved so walrus still maps them correctly.
    from concourse import hw_specs as _hw_specs
    if not getattr(bacc, "_act_tables_patched", False):
        _orig_gat = _hw_specs.get_activation_tables

        def _patched_gat(module_arch):
            tabs = _orig_gat(module_arch)
            if "natural_log_exp_and_others" in tabs:
                keep = tabs["natural_log_exp_and_others"]
                tabs = {
                    name: (fns if name == "natural_log_exp_and_others"
                           else fns - keep)
                    for name, fns in tabs.items()
                }
            return tabs

        bacc.get_activation_tables = _patched_gat
        bacc._act_tables_patched = True

    nc = bacc.Bacc("TRN2", target_bir_lowering=False, debug=False,
                   enable_asserts=False)

    f8 = mybir.dt.float8e4
    DRM = mybir.MatmulPerfMode.DoubleRow
    if fastln:
        x = nc.dram_tensor("xt", [P, ng, KD, GT * P], bf16,
                           kind="ExternalInput").ap()
        if fp8:
            w1 = nc.dram_tensor("w1", [S, P, KD // 2, KD, 2, P], f8,
                                kind="ExternalInput").ap()
            xf = nc.dram_tensor("xf8", [P, ng, KD // 2, 2, GT * P], f8,
                                kind="ExternalInput").ap()
        else:
            w1 = nc.dram_tensor("w1", [S, P, KD, KD, P], bf16,
                                kind="ExternalInput").ap()
    else:
        x = nc.dram_tensor("xt", [P, nt, KD, P], bf16,
                           kind="ExternalInput").ap()
        w1 = nc.dram_tensor("w1", [S, P, KD, D], bf16,
                            kind="ExternalInput").ap()
    fp8l2 = flags.get("fp8l2", False)
    if fp8l2:
        w2c = nc.dram_tensor("w2c", [S, P, KD // 2, 2, NCATP], f8,
                             kind="ExternalInput").ap()
    else:
        w2c = nc.dram_tensor("w2c", [S, P, KD, NCAT], bf16,
                             kind="ExternalInput").ap()
    wg1 = nc.dram_tensor("wg1", [P, KD, DH], bf16, kind="ExternalInput").ap()
    wg2 = nc.dram_tensor("wg2", [S, DH], bf16, kind="ExternalInput").ap()
    out = nc.dram_tensor("out", [nt * P, C], f32, kind="ExternalOutput").ap()

    aux_dram = {}
    for name, shape, dt_ in (("b1v", [S, D], bf16), ("g1v", [S, D], bf16),
                             ("be1v", [S, D], bf16), ("b2v", [S, NCAT], f32),
                             ("bg1v", [DH], f32), ("ggv", [DH], f32),
                             ("bgbv", [DH], f32), ("bg2v", [S], f32)):
        key = name[:-1]
        fkey = {"b1": "b1", "g1": "aff1", "be1": "aff1", "b2": "b2",
                "bg1": "bg1", "gg": "affg", "bgb": "affg", "bg2": "bg2"}[key]
        if flags[fkey]:
            aux_dram[name] = nc.dram_tensor(name, shape, dt_,
                                            kind="ExternalInput").ap()

    def bcast(dram_ap):
        # prepend a stride-0 partition axis for replicate-DMA
        return bass.AP(tensor=dram_ap.tensor, offset=dram_ap.offset,
                       ap=[[0, P]] + [list(d) for d in dram_ap.ap])

    with TileContext(nc) as tc:
        with (
            tc.tile_pool(name="wts", bufs=1) as wp,
            tc.tile_pool(name="xin", bufs=2) as xp,
            tc.tile_pool(name="hts", bufs=2) as hp,
            tc.tile_pool(name="trans", bufs=2) as trp,
            tc.tile_pool(name="work", bufs=2) as wkp,
            tc.tile_pool(name="small", bufs=6) as sp,
            tc.tile_pool(name="ps_big", bufs=2, space="PSUM") as pbig,
            tc.tile_pool(name="ps_l1", bufs=3, space="PSUM") as pl1,
            tc.tile_pool(name="ps_fg", bufs=2, space="PSUM") as pfg,
            tc.tile_pool(name="ps_tp", bufs=2, space="PSUM") as ptp,
        ):
            ident32 = wp.tile([P, P], f32, tag="id32")
            make_identity(nc, ident32)
            ident16 = wp.tile([P, P], bf16, tag="id16")
            make_identity(nc, ident16)

            def const_tile(val, cname):
                t = wp.tile([P, 1], f32, tag=cname, name=cname)
                nc.vector.memset(t, val)
                return t

            c_lneps = const_tile(LN_EPS, "c_lneps")
            c_ln5 = const_tile(float(np.log(0.5 / TEMP)), "c_ln5")
            c_eps = const_tile(EPS, "c_eps")

            w1s, w2s = [], []
            for s in range(S):
                if fastln and fp8:
                    t1 = wp.tile([P, KD // 2, KD, 2, P], f8, tag=f"w1_{s}",
                                 name=f"w1s{s}")
                elif fastln:
                    t1 = wp.tile([P, KD, KD, P], bf16, tag=f"w1_{s}",
                                 name=f"w1s{s}")
                else:
                    t1 = wp.tile([P, KD, D], bf16, tag=f"w1_{s}",
                                 name=f"w1s{s}")
                nc.sync.dma_start(out=t1, in_=w1[s])
                w1s.append(t1)
                if fp8l2:
                    t2 = wp.tile([P, KD // 2, 2, NCATP], f8, tag=f"w2_{s}",
                                 name=f"w2s{s}")
                else:
                    t2 = wp.tile([P, KD, NCAT], bf16, tag=f"w2_{s}",
                                 name=f"w2s{s}")
                nc.sync.dma_start(out=t2, in_=w2c[s])
                w2s.append(t2)
            wg1t = wp.tile([P, KD, DH], bf16, tag="wg1t")
            nc.sync.dma_start(out=wg1t, in_=wg1)
            wg2rs = []
            for s in range(S):
                t3 = wp.tile([P, DH], bf16, tag=f"wg2r{s}", name=f"wg2r{s}")
                nc.sync.dma_start(out=t3, in_=bcast(wg2[s]))
                wg2rs.append(t3)

            aux = {}
            if flags["b1"]:
                aux["b1r"] = [wp.tile([P, D], bf16, tag=f"b1r{s}", name=f"b1r{s}")
                              for s in range(S)]
                for s in range(S):
                    nc.sync.dma_start(out=aux["b1r"][s], in_=bcast(aux_dram["b1v"][s]))
            if flags["aff1"]:
                aux["g1r"] = [wp.tile([P, D], bf16, tag=f"g1r{s}", name=f"g1r{s}")
                              for s in range(S)]
                aux["be1r"] = [wp.tile([P, D], bf16, tag=f"be1r{s}", name=f"be1r{s}")
                               for s in range(S)]
                for s in range(S):
                    nc.sync.dma_start(out=aux["g1r"][s], in_=bcast(aux_dram["g1v"][s]))
                    nc.sync.dma_start(out=aux["be1r"][s], in_=bcast(aux_dram["be1v"][s]))
            if flags["b2"]:
                aux["b2r"] = [wp.tile([P, NCAT], f32, tag=f"b2r{s}", name=f"b2r{s}")
                              for s in range(S)]
                for s in range(S):
                    nc.sync.dma_start(out=aux["b2r"][s], in_=bcast(aux_dram["b2v"][s]))
            if flags["bg1"]:
                aux["bg1r"] = wp.tile([P, DH], f32, tag="bg1r", name="bg1r")
                nc.sync.dma_start(out=aux["bg1r"], in_=bcast(aux_dram["bg1v"]))
            if flags["affg"]:
                aux["ggr"] = wp.tile([P, DH], f32, tag="ggr", name="ggr")
                aux["bgbr"] = wp.tile([P, DH], f32, tag="bgbr", name="bgbr")
                nc.sync.dma_start(out=aux["ggr"], in_=bcast(aux_dram["ggv"]))
                nc.sync.dma_start(out=aux["bgbr"], in_=bcast(aux_dram["bgbv"]))
            if flags["bg2"]:
                aux["bg2r"] = wp.tile([P, S], f32, tag="bg2r", name="bg2r")
                nc.sync.dma_start(out=aux["bg2r"], in_=bcast(aux_dram["bg2v"]))

            def ln_relu(src_ps, dst, width, bias_r, gain_r, beta_r, widx):
                """dst(bf16) = relu(LN(src + bias)*gain + beta); src is PSUM fp32."""
                src2 = src_ps
                if bias_r is not None:
                    hb = wkp.tile([P, width], f32, tag=f"hb{width}",
                                  name=f"hb_{widx}")
                    nc.vector.tensor_tensor(out=hb, in0=src_ps, in1=bias_r,
                                            op=OP.add)
                    src2 = hb
                nchunks = width // 512
                st = sp.tile([P, nchunks, 6], f32, tag=f"st{width}",
                             name=f"st_{widx}")
                for j in range(nchunks):
                    nc.vector.bn_stats(out=st[:, j, :],
                                       in_=src2[:, j * 512:(j + 1) * 512])
                mv = sp.tile([P, 2], f32, tag="mv", name=f"mv_{widx}")
                nc.vector.bn_aggr(out=mv, in_=st)
                # rstd = exp(-0.5 * ln(var + eps))  (stays in one ACT table set)
                lnv = sp.tile([P, 1], f32, tag="lnv", name=f"lnv_{widx}")
                nc.scalar.activation(out=lnv, in_=mv[:, 1:2], func=AF.Ln,
                                     bias=c_lneps, scale=1.0)
                rstd = sp.tile([P, 1], f32, tag="rstd", name=f"rstd_{widx}")
                nc.scalar.activation(out=rstd, in_=lnv, func=AF.Exp,
                                     bias=0.0, scale=-0.5)
                if gain_r is None:
                    nmr = sp.tile([P, 1], f32, tag="nmr", name=f"nmr_{widx}")
                    nc.vector.tensor_scalar(out=nmr, in0=mv[:, 0:1],
                                            scalar1=rstd, scalar2=-1.0,
                                            op0=OP.mult, op1=OP.mult)
                    nc.scalar.activation(out=dst, in_=src2, func=AF.Relu,
                                         bias=nmr, scale=rstd)
                else:
                    t0 = wkp.tile([P, width], f32, tag=f"gw{width}",
                                  name=f"lnt_{widx}")
                    nc.vector.tensor_scalar(out=t0, in0=src2,
                                            scalar1=mv[:, 0:1], scalar2=rstd,
                                            op0=OP.subtract, op1=OP.mult)
                    t1 = wkp.tile([P, width], f32, tag=f"gw{width}",
                                  name=f"lnu_{widx}")
                    nc.vector.tensor_tensor(out=t1, in0=t0, in1=gain_r,
                                            op=OP.mult)
                    t2 = wkp.tile([P, width], f32, tag=f"gw{width}",
                                  name=f"lnw_{widx}")
                    nc.vector.tensor_tensor(out=t2, in0=t1, in1=beta_r,
                                            op=OP.add)
                    nc.scalar.activation(out=dst, in_=t2, func=AF.Relu)

            # ================= fast path (fastln) =========================

            def _pass_a4(g):
                """Group of GT batch tiles: gate L1 per tile + TRANSPOSED
                adapter L1 (W1' stationary, X^T moving, N=GT*128 streams).
                Produces H'^T[s] in SBUF [P(e_p), KD(e_k), GT*P] bf16."""
                xt = xp.tile([P, KD, GT * P], bf16, tag="xt", name=f"xt{g}")
                nc.sync.dma_start(out=xt, in_=x[:, g, :, :])
                if fp8:
                    xtf = xp.tile([P, KD // 2, 2, GT * P], f8, tag="xtf",
                                  name=f"xtf{g}")
                    nc.sync.dma_start(out=xtf, in_=xf[:, g])

                # gate layer 1 + LN per tile in the group
                ubs = []
                for t in range(GT):
                    u_ps = pbig.tile([P, DH], f32, tag="big", name=f"ups{g}_{t}")
                    for k in range(KD):
                        nc.tensor.matmul(
                            u_ps, lhsT=xt[:, k, t * P:(t + 1) * P],
                            rhs=wg1t[:, k, :],
                            start=(k == 0), stop=(k == KD - 1))
                    ub = wkp.tile([P, DH], bf16, tag="ub", bufs=2 * GT + 1,
                                  name=f"ub{g}_{t}")
                    ln_relu(u_ps, ub, DH, aux.get("bg1r"), aux.get("ggr"),
                            aux.get("bgbr"), f"g{g}_{t}")
                    ubs.append(ub)

                # adapter L1, transposed: out chunk = [e_p, batch], relu on evict
                hts = []
                for s in range(S):
                    ht = hp.tile([P, KD, GT * P], f8 if fp8l2 else bf16,
                                 tag=f"ht{s}", name=f"ht{g}_{s}")
                    for e in range(KD):
                        h_ps = pl1.tile([P, GT * P], f32, tag="l1",
                                        name=f"hps{g}_{s}_{e}")
                        if fp8:
                            for dd in range(KD // 2):
                                nc.tensor.matmul(
                                    h_ps, lhsT=w1s[s][:, dd, e, :, :],
                                    rhs=xtf[:, dd, :, :],
                                    start=(dd == 0), stop=(dd == KD // 2 - 1),
                                    perf_mode=DRM)
                        else:
                            for d in range(KD):
                                nc.tensor.matmul(
                                    h_ps, lhsT=w1s[s][:, d, e, :],
                                    rhs=xt[:, d, :],
                                    start=(d == 0), stop=(d == KD - 1))
                        # relu happens during the single PSUM->SBUF eviction;
                        # alternate engines to balance ACT/DVE load. With
                        # fp8 L2 the eviction also rescales into e4m3 range
                        # (global scale cancels through the cosine).
                        if (s + e) % 2 == 0:
                            nc.scalar.activation(out=ht[:, e, :], in_=h_ps,
                                                 func=AF.Relu,
                                                 scale=HSC if fp8l2 else 1.0)
                        else:
                            nc.vector.tensor_scalar(out=ht[:, e, :], in0=h_ps,
                                                    scalar1=HSC if fp8l2 else 1.0,
                                                    scalar2=0.0,
                                                    op0=OP.mult, op1=OP.max)
                    hts.append(ht)

                return dict(g=g, ubs=ubs, hts=hts)

            def _pass_b_fast(st, t):
                g, ub, hts = st["g"], st["ubs"][t], st["hts"]
                i_ = g * GT + t
                bsl = slice(t * P, (t + 1) * P)

                # gate logits via DVE dot products
                g2 = sp.tile([P, S], f32, tag="g2", name=f"g2{i_}")
                gscrap = wkp.tile([P, DH], bf16, tag="gscrap",
                                  name=f"gscrap{i_}")
                for j in range(S):
                    nc.vector.scalar_tensor_tensor(
                        out=gscrap, in0=ub, scalar=1.0, in1=wg2rs[j],
                        op0=OP.bypass, op1=OP.mult,
                        accum_out=g2[:, j:j + 1])
                gsrc = g2
                if flags["bg2"]:
                    gb_sb = sp.tile([P, S], f32, tag="gbsb", name=f"gbsb{i_}")
                    nc.vector.tensor_tensor(out=gb_sb, in0=g2, in1=aux["bg2r"],
                                            op=OP.add)
                    gsrc = gb_sb
                mg = sp.tile([P, 1], f32, tag="mg", name=f"mg{i_}")
                nc.vector.tensor_reduce(out=mg, in_=gsrc, axis=AX.X, op=OP.max)
                nmg = sp.tile([P, 1], f32, tag="nmg", name=f"nmg{i_}")
                nc.vector.tensor_scalar(out=nmg, in0=mg, scalar1=-1.0,
                                        scalar2=None, op0=OP.mult)
                eg = sp.tile([P, S], f32, tag="eg", name=f"eg{i_}")
                sg = sp.tile([P, 1], f32, tag="sg", name=f"sg{i_}")
                nc.scalar.activation(out=eg, in_=gsrc, func=AF.Exp, bias=nmg,
                                     scale=1.0, accum_out=sg)
                rg = sp.tile([P, 1], f32, tag="rg", name=f"rg{i_}")
                nc.vector.reciprocal(out=rg, in_=sg)

                acc = None
                for s in range(S):
                    fg = pfg.tile([P, NCATP if fp8l2 else NCAT], f32,
                                  tag="fg", name=f"fg{i_}_{s}")
                    if fp8l2:
                        for ee in range(KD // 2):
                            nc.tensor.matmul(
                                fg, lhsT=hts[s][:, 2 * ee:2 * ee + 2, bsl],
                                rhs=w2s[s][:, ee, :, :],
                                start=(ee == 0), stop=(ee == KD // 2 - 1),
                                perf_mode=DRM)
                    else:
                        for k in range(KD):
                            nc.tensor.matmul(fg, lhsT=hts[s][:, k, bsl],
                                             rhs=w2s[s][:, k, :],
                                             start=(k == 0), stop=(k == KD - 1))
                    # ---- row norm of F ----
                    sq = wkp.tile([P, NF], f32, tag="sq", name=f"sq{i_}_{s}")
                    n2 = sp.tile([P, 1], f32, tag="n2", name=f"n2{i_}_{s}")
                    nc.scalar.activation(out=sq, in_=fg[:, :NF],
                                         func=AF.Square, accum_out=n2)
                    n2c = sp.tile([P, 1], f32, tag="n2c", name=f"n2c{i_}_{s}")
                    nc.vector.tensor_scalar(out=n2c, in0=n2, scalar1=1e-16,
                                            scalar2=None, op0=OP.max)
                    lnn = sp.tile([P, 1], f32, tag="lnn", name=f"lnn{i_}_{s}")
                    nc.scalar.activation(out=lnn, in_=n2c, func=AF.Ln)
                    # rn5 = 5 / sqrt(n2c) = exp(-0.5*ln(n2c) + ln 5)
                    rn5 = sp.tile([P, 1], f32, tag="rn5", name=f"rn5{i_}_{s}")
                    nc.scalar.activation(out=rn5, in_=lnn, func=AF.Exp,
                                         bias=c_ln5, scale=-0.5)
                    logit = sp.tile([P, C], f32, tag="logit",
                                    name=f"lg{i_}_{s}")
                    nc.vector.tensor_scalar(out=logit, in0=fg[:, NF:NF + C],
                                            scalar1=rn5, scalar2=None,
                                            op0=OP.mult)
                    # ---- softmax over classes ----
                    ml_ = sp.tile([P, 1], f32, tag="ml", name=f"ml{i_}_{s}")
                    nc.vector.tensor_reduce(out=ml_, in_=logit, axis=AX.X,
                                            op=OP.max)
                    nml = sp.tile([P, 1], f32, tag="nml", name=f"nml{i_}_{s}")
                    nc.vector.tensor_scalar(out=nml, in0=ml_, scalar1=-1.0,
                                            scalar2=None, op0=OP.mult)
                    es = sp.tile([P, C], f32, tag="es", name=f"es{i_}_{s}")
                    se = sp.tile([P, 1], f32, tag="se", name=f"se{i_}_{s}")
                    nc.scalar.activation(out=es, in_=logit, func=AF.Exp,
                                         bias=nml, scale=1.0, accum_out=se)
                    rse = sp.tile([P, 1], f32, tag="rse", name=f"rse{i_}_{s}")
                    nc.vector.reciprocal(out=rse, in_=se)
                    q = sp.tile([P, 1], f32, tag="q", name=f"q{i_}_{s}")
                    nc.vector.tensor_scalar(out=q, in0=eg[:, s:s + 1],
                                            scalar1=rse, scalar2=rg,
                                            op0=OP.mult, op1=OP.mult)
                    acc2 = sp.tile([P, C], f32, tag="acc", name=f"acc{i_}_{s}")
                    if acc is None:
                        nc.vector.tensor_scalar(out=acc2, in0=es, scalar1=q,
                                                scalar2=None, op0=OP.mult)
                    else:
                        nc.vector.scalar_tensor_tensor(out=acc2, in0=es,
                                                       scalar=q, in1=acc,
                                                       op0=OP.mult, op1=OP.add)
                    acc = acc2

                ot = sp.tile([P, C], f32, tag="ot", name=f"ot{i_}")
                nc.scalar.activation(out=ot, in_=acc, func=AF.Ln, bias=c_eps,
                                     scale=1.0)
                nc.sync.dma_start(out=out[i_ * P:(i_ + 1) * P, :], in_=ot)

            # ================= generic path (original) ====================

            def _pass_a(i, i_):
                xt = xp.tile([P, KD, P], bf16, tag="xt", bufs=3,
                             name=f"xt{i}")
                nc.sync.dma_start(out=xt, in_=x[:, i_, :, :])

                u_ps = pbig.tile([P, DH], f32, tag="big", name=f"ups{i}")
                for k in range(KD):
                    nc.tensor.matmul(u_ps, lhsT=xt[:, k, :], rhs=wg1t[:, k, :],
                                     start=(k == 0), stop=(k == KD - 1))
                ub = wkp.tile([P, DH], bf16, tag="ub", bufs=3, name=f"ub{i}")
                ln_relu(u_ps, ub, DH, aux.get("bg1r"), aux.get("ggr"),
                        aux.get("bgbr"), f"g{i}")

                hns = []
                for s in range(S):
                    h_ps = pbig.tile([P, D], f32, tag="big", name=f"hps{i}_{s}")
                    for k in range(KD):
                        for n in range(2):
                            nc.tensor.matmul(
                                h_ps[:, n * 512:(n + 1) * 512],
                                lhsT=xt[:, k, :],
                                rhs=w1s[s][:, k, n * 512:(n + 1) * 512],
                                start=(k == 0), stop=(k == KD - 1))
                    hn = wkp.tile([P, D], bf16, tag="hn", bufs=10,
                                  name=f"hn{i}_{s}")
                    ln_relu(h_ps, hn, D,
                            aux["b1r"][s] if flags["b1"] else None,
                            aux["g1r"][s] if flags["aff1"] else None,
                            aux["be1r"][s] if flags["aff1"] else None,
                            f"h{i}_{s}")
                    hns.append(hn)

                return dict(i=i, i_=i_, ub=ub, hns=hns)

            def _pass_b(st):
                i, i_, ub, hns = st["i"], st["i_"], st["ub"], st["hns"]

                def transp_chunk(s, k, tp4s, hts_dst):
                    half, j = k // 4, k % 4
                    if j == 0:
                        tp4s[half] = ptp.tile([P, 4, P], bf16, tag="tp",
                                              name=f"htp{i}_{s}_{half}")
                    nc.tensor.transpose(out=tp4s[half][:, j, :],
                                        in_=hns[s][:, k * P:(k + 1) * P],
                                        identity=ident16)
                    if j == 3:
                        hth = trp.tile([P, 4, P], bf16, tag=f"ht{half}",
                                       name=f"ht{i}_{s}_{half}")
                        if half == 0:
                            nc.vector.tensor_copy(out=hth, in_=tp4s[half])
                        else:
                            nc.scalar.copy(out=hth, in_=tp4s[half])
                        hts_dst[half] = hth

                hts_cur = [None, None]
                tp4s = [None, None]
                for k in range(KD):
                    transp_chunk(0, k, tp4s, hts_cur)

                g2 = sp.tile([P, S], f32, tag="g2", name=f"g2{i}")
                gscrap = wkp.tile([P, DH], bf16, tag="gscrap",
                                  name=f"gscrap{i}")
                for j in range(S):
                    nc.vector.scalar_tensor_tensor(
                        out=gscrap, in0=ub, scalar=1.0, in1=wg2rs[j],
                        op0=OP.bypass, op1=OP.mult,
                        accum_out=g2[:, j:j + 1])
                gsrc = g2
                if flags["bg2"]:
                    gb_sb = sp.tile([P, S], f32, tag="gbsb", name=f"gbsb{i}")
                    nc.vector.tensor_tensor(out=gb_sb, in0=g2, in1=aux["bg2r"],
                                            op=OP.add)
                    gsrc = gb_sb
                mg = sp.tile([P, 1], f32, tag="mg", name=f"mg{i}")
                nc.vector.tensor_reduce(out=mg, in_=gsrc, axis=AX.X, op=OP.max)
                nmg = sp.tile([P, 1], f32, tag="nmg", name=f"nmg{i}")
                nc.vector.tensor_scalar(out=nmg, in0=mg, scalar1=-1.0,
                                        scalar2=None, op0=OP.mult)
                eg = sp.tile([P, S], f32, tag="eg", name=f"eg{i}")
                sg = sp.tile([P, 1], f32, tag="sg", name=f"sg{i}")
                nc.scalar.activation(out=eg, in_=gsrc, func=AF.Exp, bias=nmg,
                                     scale=1.0, accum_out=sg)
                rg = sp.tile([P, 1], f32, tag="rg", name=f"rg{i}")
                nc.vector.reciprocal(out=rg, in_=sg)

                acc = None
                for s in range(S):
                    hts_next = [None, None]
                    tp4s = [None, None]
                    fg = pfg.tile([P, NCAT], f32, tag="fg", name=f"fg{i}_{s}")
                    for k in range(KD):
                        nc.tensor.matmul(fg, lhsT=hts_cur[k // 4][:, k % 4, :],
                                         rhs=w2s[s][:, k, :],
                                         start=(k == 0), stop=(k == KD - 1))
                        if s + 1 < S:
                            transp_chunk(s + 1, k, tp4s, hts_next)
                    fsrc = fg
                    if flags["b2"]:
                        fb = wkp.tile([P, NCAT], f32, tag="fb",
                                      name=f"fb{i}_{s}")
                        nc.vector.tensor_tensor(out=fb, in0=fg,
                                                in1=aux["b2r"][s], op=OP.add)
                        fsrc = fb
                    sq = wkp.tile([P, NF], f32, tag="sq", name=f"sq{i}_{s}")
                    n2 = sp.tile([P, 1], f32, tag="n2", name=f"n2{i}_{s}")
                    nc.scalar.activation(out=sq, in_=fsrc[:, :NF],
                                         func=AF.Square, accum_out=n2)
                    n2c = sp.tile([P, 1], f32, tag="n2c", name=f"n2c{i}_{s}")
                    nc.vector.tensor_scalar(out=n2c, in0=n2, scalar1=1e-16,
                                            scalar2=None, op0=OP.max)
                    lnn = sp.tile([P, 1], f32, tag="lnn", name=f"lnn{i}_{s}")
                    nc.scalar.activation(out=lnn, in_=n2c, func=AF.Ln)
                    rn5 = sp.tile([P, 1], f32, tag="rn5", name=f"rn5{i}_{s}")
                    nc.scalar.activation(out=rn5, in_=lnn, func=AF.Exp,
                                         bias=c_ln5, scale=-0.5)
                    logit = sp.tile([P, C], f32, tag="logit",
                                    name=f"lg{i}_{s}")
                    nc.vector.tensor_scalar(out=logit, in0=fsrc[:, NF:NF + C],
                                            scalar1=rn5, scalar2=None,
                                            op0=OP.mult)
                    ml_ = sp.tile([P, 1], f32, tag="ml", name=f"ml{i}_{s}")
                    nc.vector.tensor_reduce(out=ml_, in_=logit, axis=AX.X,
                                            op=OP.max)
                    nml = sp.tile([P, 1], f32, tag="nml", name=f"nml{i}_{s}")
                    nc.vector.tensor_scalar(out=nml, in0=ml_, scalar1=-1.0,
                                            scalar2=None, op0=OP.mult)
                    es = sp.tile([P, C], f32, tag="es", name=f"es{i}_{s}")
                    se = sp.tile([P, 1], f32, tag="se", name=f"se{i}_{s}")
                    nc.scalar.activation(out=es, in_=logit, func=AF.Exp,
                                         bias=nml, scale=1.0, accum_out=se)
                    rse = sp.tile([P, 1], f32, tag="rse", name=f"rse{i}_{s}")
                    nc.vector.reciprocal(out=rse, in_=se)
                    q = sp.tile([P, 1], f32, tag="q", name=f"q{i}_{s}")
                    nc.vector.tensor_scalar(out=q, in0=eg[:, s:s + 1],
                                            scalar1=rse, scalar2=rg,
                                            op0=OP.mult, op1=OP.mult)
                    acc2 = sp.tile([P, C], f32, tag="acc", name=f"acc{i}_{s}")
                    if acc is None:
                        nc.vector.tensor_scalar(out=acc2, in0=es, scalar1=q,
                                                scalar2=None, op0=OP.mult)
                    else:
                        nc.vector.scalar_tensor_tensor(out=acc2, in0=es,
                                                       scalar=q, in1=acc,
                                                       op0=OP.mult, op1=OP.add)
                    acc = acc2
                    hts_cur = hts_next

                ot = sp.tile([P, C], f32, tag="ot", name=f"ot{i}")
                nc.scalar.activation(out=ot, in_=acc, func=AF.Ln, bias=c_eps,
                                     scale=1.0)
                nc.sync.dma_start(out=out[i_ * P:(i_ + 1) * P, :], in_=ot)

            def _emit_all():
                if fastln:
                    prev = None
                    for g in range(ng):
                        st = _pass_a4(g)
                        if prev is not None:
                            for t in range(GT):
                                _pass_b_fast(prev, t)
                        prev = st
                    for t in range(GT):
                        _pass_b_fast(prev, t)
                else:
                    prev = None
                    for i_ in range(nt):
                        st = _pass_a(i_, i_)
                        if prev is not None:
                            _pass_b(prev)
                        prev = st
                    _pass_b(prev)

            if repeat == 1:
                _emit_all()
            else:
                with tc.For_i(0, repeat, 1):
                    _emit_all()

    nc.compile()
    return nc


# --------------------------------------------------------------------------
# host wrapper
# --------------------------------------------------------------------------

def _get_program(flags, nt=NT, repeat=1):
    key = (tuple(sorted(flags.items())), nt, repeat)
    if key not in _CACHE:
        _CACHE[key] = _build(flags, nt, repeat)
    return _CACHE[key]


def _in_maps(arrays):
    xt_all = arrays.pop("_xt_percore")
    xf8_all = arrays.pop("_xf8_percore", None)
    in_maps = []
    for cidx in range(NCORES):
        m = dict(arrays)
        m["xt"] = np.ascontiguousarray(xt_all[cidx])
        if xf8_all is not None:
            m["xf8"] = np.ascontiguousarray(xf8_all[cidx])
        in_maps.append(m)
    return in_maps


def _run(inputs, trace=False):
    from concourse import bass_utils

    X, arrays, flags = _host_prepare(inputs)
    nc = _get_program(flags)
    in_maps = _in_maps(arrays)
    res = bass_utils.run_bass_kernel_spmd(nc, in_maps,
                                          core_ids=list(range(NCORES)),
                                          trace=trace)
    outp = np.concatenate([r["out"] for r in res.results], axis=0)
    return outp, res


def kernel(**inputs):
    outp, _ = _run(inputs, trace=False)
    return outp
